# revision 33
# baseline (speedup 1.0000x reference)
"""Sparse attention mixer (B=2,S=2048,D=1024,H=16,window=256 causal-banded)
on 8 trn2 NeuronCores.

Sharding: data-parallel over batch (2) x tensor-parallel over head groups (4).
Core c handles batch c//4, heads [4*(c%4), 4*(c%4)+4). Each core computes its
qkv projection slice, banded attention for its 4 heads, and a partial
out-projection over its 256 local dims; the host sums the 4 partials per batch
and adds the output bias.

Mask structure: mask[i,j] = 0 if j <= i+256 else -1e9  (causal OR |i-j|<=256,
clamped). Per 128-row query block qi, key blocks 0..qi+1 are fully allowed,
block qi+2 is lower-triangular (a<=b in transposed [sk,sq] layout), blocks
>qi+2 fully masked (skipped).

Schedule: PE-busy is the binding resource (~128us of bf16 streaming). The
scalar engine's exp stream paces the attention loops, so deferred projection
work (fillers) keeps the PE fed; the Tile scheduler hoists ready work into
any PE slack. Three structural choices matter:
 - all DRAM inputs are pre-swizzled on the host so every DMA line is 2-8KB
   contiguous per partition (x arrives s-block-major at 8KB/partition/line);
 - the filler matmuls accumulate in their own PSUM ring ('fill' tag) so a
   drained filler never stalls the scores/exp psum rotation;
 - softmax normalization (1/d = exp(-ln d) on ACT + partition broadcast) is
   chunk-level except the last chunk, where it is per-pair; the final pair's
   broadcast runs on the PE (K=1 f32r matmul) and its multiplies are emitted
   per 128-column m-block so each tail out_proj matmul unblocks as soon as
   its slice is normalized. Output is staged/DMA'd as bf16 (host upcasts).
"""

import sys
import types

import numpy as np

B, S, D, H = 2, 2048, 1024, 16
HD = 64          # head dim
HPC = 4          # heads per core
DL = HPC * HD    # 256 local dims per core
NCORES = 8
P = 128
NEG = np.float32(-1.0e9)
SCALE = float(HD) ** -0.5

# knobs for test harness
TRACE = False
TRACE_CORES = None
LAST_RESULTS = None

_MODULE_CACHE = {}


def _install_ntff_shim():
    """antenv.axon_hooks is absent in this image; register the NTFF profile
    hook via ctypes against the axon PJRT .so so trace=True works."""
    if 'antenv.axon_hooks' in sys.modules:
        return
    hook = None
    try:
        from trn_agent_boot.trn_boot import _ntff_profile_via_ctypes
        hook = _ntff_profile_via_ctypes('/opt/axon/libaxon_pjrt.so')
    except Exception:
        hook = None
    m = types.ModuleType('antenv.axon_hooks')
    m.get_axon_ntff_profile_hook = lambda: hook
    m.set_axon_ntff_profile_hook = lambda h: None
    sys.modules['antenv.axon_hooks'] = m


def _build_module():
    import concourse.mybir as mybir
    import concourse.tile as tile
    from concourse import bacc
    from concourse.bass import ts

    dt = mybir.dt
    f32 = dt.float32
    f32r = dt.float32r
    bf16 = dt.bfloat16
    AF = mybir.ActivationFunctionType

    ND = D // P      # 8 d-chunks
    NB = S // P      # 16 s-blocks of 128

    nc = bacc.Bacc('TRN2', target_bir_lowering=False, debug=False,
                   num_devices=NCORES)

    # all inputs pre-swizzled on host: per-partition-contiguous lines
    xs = nc.dram_tensor('xs', [P, 4, ND, 512], bf16, kind='ExternalInput').ap()
    wqs = nc.dram_tensor('wqs', [P, ND, DL], bf16, kind='ExternalInput').ap()
    wks = nc.dram_tensor('wks', [P, ND, DL], bf16, kind='ExternalInput').ap()
    wvs = nc.dram_tensor('wvs', [P, ND, DL], bf16, kind='ExternalInput').ap()
    wos = nc.dram_tensor('wos', [P, 2, D], bf16, kind='ExternalInput').ap()
    bq2 = nc.dram_tensor('bq2', [P, 2], f32, kind='ExternalInput').ap()
    bk2 = nc.dram_tensor('bk2', [P, 2], f32, kind='ExternalInput').ap()
    bvrow = nc.dram_tensor('bvrow', [1, DL], f32, kind='ExternalInput').ap()
    mask01 = nc.dram_tensor('mask01', [P, P], bf16, kind='ExternalInput').ap()
    onescol = nc.dram_tensor('onescol', [P, 16, 1], bf16,
                             kind='ExternalInput').ap()
    out = nc.dram_tensor('out', [S, D], bf16, kind='ExternalOutput').ap()

    def r(ap):
        return ap

    def act_raw(out_ap, in_ap, func, scale=1.0):
        # raw InstActivation (out = func(scale*in)); bypasses dtype checks
        # for f32r outputs. ins = [in, bias, scale, alpha]
        eng = nc.scalar
        ins = [eng.lower_ap(in_ap),
               mybir.ImmediateValue(dtype=f32, value=0.0),
               mybir.ImmediateValue(dtype=f32, value=float(scale)),
               mybir.ImmediateValue(dtype=f32, value=0.0)]
        eng.add_instruction(mybir.InstActivation(
            name=nc.get_next_instruction_name(),
            func=func, ins=ins, outs=[eng.lower_ap(out_ap)]))

    with tile.TileContext(nc) as tc:
        with (
            tc.tile_pool(name='const', bufs=1) as cpool,
            tc.tile_pool(name='wp', bufs=1) as wpool,
            tc.tile_pool(name='persist', bufs=1) as ppool,
            tc.tile_pool(name='expp', bufs=6) as epool,
            tc.tile_pool(name='rp', bufs=2) as rpool,
            tc.tile_pool(name='ostage', bufs=8) as opool,
            tc.tile_pool(name='mm', bufs=2, space='PSUM') as mmp,
            tc.tile_pool(name='fill', bufs=2, space='PSUM') as fpsp,
            tc.tile_pool(name='avo', bufs=2, space='PSUM') as avop,
        ):
            # ---------------- warmup: PE junk matmuls from ~t=0 ------------
            # HAM opens the PE clock gate only after ~3.4us of sustained
            # activity; keep the PE busy from the first instruction so the
            # real work (arriving with the DMAs at ~10us) runs at 2.4GHz.
            wmt = cpool.tile([P, 512], f32, name='wmt')
            nc.vector.memset(wmt[:], 1.0)
            wps = avop.tile([P, 512], f32, name='warm_ps', tag='avo')
            nc.tensor.matmul(wps[:], r(wmt[:, 0:P]), r(wmt[:]),
                             start=True, stop=False)
            for i in range(6):
                nc.tensor.matmul(wps[:, 0:P], r(wmt[:, 0:P]), r(wmt[:, 0:P]),
                                 start=False, stop=(i == 5))

            # ---------------- input DMAs -----------------------------------
            # The three DGE queues deliver ~equal bandwidth, so pieces are
            # striped round-robin across sync/scalar/gpsimd in EXACT
            # consumption order (wk, x0, x1, wv, wq, x2, x3, wo): arrival
            # order then matches the projection prefix and nothing
            # head-of-line blocks on a later transfer.
            dq = [nc.sync, nc.scalar, nc.gpsimd]
            dcnt = [0]

            def dma(dst, src, q=None):
                (dq[dcnt[0] % 3] if q is None else q).dma_start(dst, src)
                if q is None:
                    dcnt[0] += 1

            wk_sb = wpool.tile([P, ND, DL], bf16, name='wk_sb')
            xt = ppool.tile([P, 4, ND, 512], bf16, name='xt')
            wv_sb = wpool.tile([P, ND, DL], bf16, name='wv_sb')
            wq_sb = wpool.tile([P, ND, DL], bf16, name='wq_sb')
            wo_sb = wpool.tile([P, 2, D], bf16, name='wo_sb')
            dma(wk_sb[:], wks[:])
            # preload the one activation table that covers every func used
            # (exp, ln, copy) so no implicit table swaps are ever inserted
            try:
                from concourse.hw_specs import get_activation_tables
                _set_id = list(get_activation_tables(nc.m.arch)).index(
                    'natural_log_exp_and_others')
            except Exception:
                _set_id = 6
            nc.scalar.add_instruction(mybir.InstLoadActFuncSet(
                name=nc.get_next_instruction_name(),
                act_func_set_id=_set_id, ins=[], outs=[]))
            for j in range(2):
                dma(xt[:, 0, 4 * j:4 * j + 4], xs[:, 0, 4 * j:4 * j + 4])
            bk_sb = cpool.tile([P, 2], f32, name='bk_sb')
            dma(bk_sb[:], bk2, q=nc.scalar)
            for j in range(2):
                dma(xt[:, 1, 4 * j:4 * j + 4], xs[:, 1, 4 * j:4 * j + 4])
            dma(wv_sb[:], wvs[:])
            dma(wq_sb[:], wqs[:])
            bq_sb = cpool.tile([P, 2], f32, name='bq_sb')
            dma(bq_sb[:], bq2, q=nc.scalar)
            bv_sb = cpool.tile([1, DL], f32, name='bv_sb')
            dma(bv_sb[:], bvrow, q=nc.gpsimd)
            ones16_sb = cpool.tile([P, 16], bf16, name='ones16_sb')
            dma(ones16_sb[:], onescol.rearrange('p n o -> p (n o)'),
                q=nc.sync)
            for j in range(2):
                dma(xt[:, 2, 4 * j:4 * j + 4], xs[:, 2, 4 * j:4 * j + 4])
            m01_sb = cpool.tile([P, P], bf16, name='m01_sb')
            dma(m01_sb[:], mask01, q=nc.gpsimd)
            for j in range(2):
                dma(xt[:, 3, 4 * j:4 * j + 4], xs[:, 3, 4 * j:4 * j + 4])
            dma(wo_sb[:], wos[:])
            # V bias broadcast to all partitions (added during V psum->sbuf)
            bvb_sb = cpool.tile([P, DL], f32, name='bvb_sb')
            nc.gpsimd.partition_broadcast(bvb_sb[:], bv_sb[:])
            # Block-ones stationary for the PE norm-broadcast matmuls:
            # out[0:64]   = moving row 0    (pair's hi=0 1/d row)
            # out[64:128] = moving row 32   (pair's hi=1 1/d row)
            # Replicated at base rows 0 and 64 so either head pair's packed
            # gr rows can be the moving operand with matching base partition.
            ones2f = cpool.tile([97, P], f32, name='ones2f')
            nc.vector.memset(ones2f[:], 0.0)
            for b0 in (0, 64):
                nc.vector.memset(ones2f[b0:b0 + 1, 0:64], 1.0)
                nc.vector.memset(ones2f[b0 + 32:b0 + 33, 64:128], 1.0)
            ones2r = cpool.tile([97, P], f32r, name='ones2r')
            act_raw(ones2r[:], ones2f[:], AF.Copy)

            # ---------------- persistent intermediates ----------------
            # pair t holds heads {2t, 2t+1} stacked along partitions (64 each)
            qT_sb = [ppool.tile([P, S], bf16, name=f'qT{t}') for t in range(2)]
            kT_sb = [ppool.tile([P, S], bf16, name=f'kT{t}') for t in range(2)]
            # V blocks: per s-block, per head: 64 V columns + 1 ones column
            v_sb = ppool.tile([P, NB, HPC * (HD + 1)], bf16, name='v_sb')
            # attn outT pairs: partitions = 128 local dims of pair t, free = s
            aoT_sb = [ppool.tile([P, S], bf16, name=f'aoT{t}') for t in range(2)]

            # ---------------- projection group emitters ----------------
            # qT/kT: per (q/k, pair t, 512-wide s-range): psum [128, 512], 8
            # c-chunk accumulation MMs, then scale+bias on the psum->sbuf
            # move. Fillers accumulate in their own 'fill' psum ring so they
            # never stall the scores/exp rotation.
            def emit_qk(which, t, s0):
                wsb, bsb, dst, scale = (
                    (wq_sb, bq_sb, qT_sb, SCALE) if which == 'q' else
                    (wk_sb, bk_sb, kT_sb, 1.0))
                sb = s0 // 512
                ps = fpsp.tile([P, 512], f32, name=f'{which}ps{t}_{s0}',
                               tag='fill')
                for c in range(ND):
                    nc.tensor.matmul(
                        ps[:], r(wsb[:, c, ts(t, P)]), r(xt[:, sb, c, :]),
                        start=(c == 0), stop=(c == ND - 1))
                nc.vector.tensor_scalar(
                    out=dst[t][:, s0:s0 + 512], in0=ps[:],
                    scalar1=scale, scalar2=bsb[:, t:t + 1],
                    op0=mybir.AluOpType.mult,
                    op1=mybir.AluOpType.add)

            # V: out [128(s), 256(o)] per s-block; bias added on the
            # psum->sbuf move as one strided add (dst skips ones columns)
            def emit_v(sbb):
                sb, qq = divmod(sbb, 4)
                vps = fpsp.tile([P, DL], f32, name=f'v_ps{sbb}', tag='fill')
                for c in range(ND):
                    nc.tensor.matmul(
                        vps[:], r(xt[:, sb, c, ts(qq, P)]), r(wv_sb[:, c, :]),
                        start=(c == 0), stop=(c == ND - 1))
                dst = v_sb[:, sbb, :].rearrange('p (h e) -> p h e', e=HD + 1)
                nc.vector.tensor_add(
                    dst[:, :, 0:HD],
                    vps.rearrange('p (h e) -> p h e', e=HD),
                    bvb_sb.rearrange('p (h e) -> p h e', e=HD))

            # out_proj for s-block m, d-half n: psum [128, 512], 2 MMs; the
            # psum->sbuf stage runs on ACT or DVE; output staged as bf16.
            oq_cnt = [0]
            oq = [nc.sync, nc.scalar, nc.gpsimd]

            def emit_oproj(m, n, stage_eng):
                ops = fpsp.tile([P, 512], f32, name=f'o_ps{m}_{n}', tag='fill')
                for t in range(2):
                    nc.tensor.matmul(ops[:],
                                     r(aoT_sb[t][:, ts(m, P)]),
                                     r(wo_sb[:, t, ts(n, 512)]),
                                     start=(t == 0), stop=(t == 1))
                ost = opool.tile([P, 512], bf16, name=f'ost{m}_{n}', tag='ost')
                if stage_eng == 'act':
                    nc.scalar.activation(ost[:], ops[:], AF.Copy)
                else:
                    nc.vector.tensor_copy(ost[:], ops[:])
                q = oq[oq_cnt[0] % len(oq)]
                oq_cnt[0] += 1
                q.dma_start(out[ts(m, P), ts(n, 512)], ost[:])

            # ---------------- deferred-work (filler) queue ----------------
            fillers = []

            def drain(n=1):
                for _ in range(n):
                    if fillers:
                        fillers.pop(0)()

            # ---------------- attention chunk ----------------
            def emit_att(c, fill_at, final=False):
                """Attention for query chunk c (s columns [512c, 512c+512)).
                fill_at[t] = set of kb indices after whose scores+AV emission
                one filler group is drained. Normalization: 1/d = exp(-ln d)
                on ACT over a packed sums tile, partition-broadcast, multiply
                into aoT. final=True (last chunk): per-pair norm; pair t=1's
                broadcast runs on the PE (K=1 f32r matmul into the then-idle
                'mm' psum ring) and its multiplies are emitted per 128-col
                m-block so the tail out_proj pipeline starts ASAP."""
                aou = []
                if not final:
                    g = rpool.tile([97, 512], f32, name=f'g{c}', tag='g',
                                   bufs=2)
                    nc.vector.memset(g[:], 1.0)
                for t in range(2):      # head pair; heads 2t (rows 0:64), 2t+1
                    last = final and t == 1
                    kb_max = min(NB, 4 * c + 6)   # key blocks 0..kb_max-1
                    avo = [avop.tile([HD + 1, 512], f32,
                                     name=f'avo{c}_{2 * t + hi}', tag='avo')
                           for hi in range(2)]
                    # software-pipelined: scores(kb) pair emitted back-to-back
                    # into one 2-bank psum tile (halves: hi=0 -> [0:512],
                    # hi=1 -> [512:1024]; distinct PE row groups overlap),
                    # one merged exp per kb, AV(kb-2) after scores(kb) so the
                    # AV matmul never waits on exp latency (2 slots of slack)
                    def emit_av(pend, last_av):
                        pet, pn0, pkb = pend
                        for hi in range(2):
                            h = 2 * t + hi
                            nc.tensor.matmul(
                                avo[hi][:, pn0:],
                                r(v_sb[:, pkb,
                                       h * (HD + 1):(h + 1) * (HD + 1)]),
                                r(pet[:, 512 * hi + pn0:512 * (hi + 1)]),
                                start=(pkb == 0), stop=last_av,
                                skip_group_check=True)

                    pends = []
                    for kb in range(kb_max):
                        z = max(0, kb - 4 * c - 2)   # fully-masked sub-blocks
                        n0 = P * z
                        lb = kb - 2 - 4 * c          # banded sub-block index
                        sps = mmp.tile([P, 1024], f32,
                                       name=f's_ps{c}_{t}_{kb}', tag='mm')
                        for hi in range(2):
                            nc.tensor.matmul(
                                sps[:, 512 * hi + n0:512 * (hi + 1)],
                                r(kT_sb[t][64 * hi:64 * hi + 64, ts(kb, P)]),
                                r(qT_sb[t][64 * hi:64 * hi + 64,
                                           512 * c + n0:512 * (c + 1)]),
                                start=True, stop=True)
                        et = epool.tile([P, 1024], bf16,
                                        name=f'exp{c}_{t}_{kb}', tag='exp')
                        spsv = sps.rearrange('p (u q) -> p u q', u=2)
                        etv = et.rearrange('p (u q) -> p u q', u=2)
                        nc.scalar.activation(etv[:, :, n0:], spsv[:, :, n0:],
                                             AF.Exp)
                        if 0 <= lb < 4:
                            nc.vector.tensor_mul(
                                etv[:, :, 128 * lb:128 * lb + 128],
                                etv[:, :, 128 * lb:128 * lb + 128],
                                m01_sb[:, None, :].broadcast_to([P, 2, P]))
                        pends.append((et, n0, kb))
                        if len(pends) > 2:
                            emit_av(pends.pop(0), False)
                        if kb in fill_at[t]:
                            drain()
                    while len(pends) > 1:
                        emit_av(pends.pop(0), False)
                    emit_av(pends.pop(0), True)
                    # gather the sums rows straight from psum BEFORE the big
                    # staging copies, so 1/d = exp(-ln d) overlaps them.
                    # Final pair: row 0 on ACT (free by then), row 1 on DVE.
                    if final:
                        g = rpool.tile([33, 512], f32, name=f'g{c}_{t}',
                                       tag='g', bufs=2)
                        nc.vector.memset(g[:], 1.0)
                    for hi in range(2):
                        row = 32 * hi if final else 32 * (2 * t + hi)
                        if last and hi == 0:
                            nc.scalar.activation(g[row:row + 1, :],
                                                 avo[hi][64:65, :], AF.Copy)
                        else:
                            nc.vector.tensor_copy(g[row:row + 1, :],
                                                  avo[hi][64:65, :])
                    if last:
                        # ~1.4us of tiny junk matmuls that READ g (so the
                        # scheduler cannot hoist them before the last AV):
                        # they keep the HAM clock gate open through the
                        # final Ln/Exp chain, else the tail out_proj matmuls
                        # all run at 1.2GHz
                        jps = fpsp.tile([P, 64], f32, name='jps', tag='fill')
                        for i in range(6):
                            nc.tensor.matmul(jps[:], r(wmt[0:33, 0:P]),
                                             r(g[:, 0:64]),
                                             start=(i == 0), stop=(i == 5))
                    # release avo: stage unnormalized result to SBUF
                    pa = []
                    for hi in range(2):
                        ao = rpool.tile([HD, 512], f32,
                                        name=f'aou{c}_{2 * t + hi}', tag='aou',
                                        bufs=8)
                        if last and hi == 0:
                            nc.scalar.activation(ao[:], avo[hi][0:HD, :],
                                                 AF.Copy)
                        else:
                            nc.vector.tensor_copy(ao[:], avo[hi][0:HD, :])
                        aou.append(ao)
                        pa.append(ao)
                    if final:
                        # per-pair norm so t=0's norm overlaps t=1's attention
                        gl = rpool.tile([33, 512], f32, name=f'gl{c}_{t}',
                                        tag='gl', bufs=2)
                        act_raw(gl[:], g[:], AF.Ln)
                        gr = rpool.tile([33, 512], f32r,
                                        name=f'gr{c}_{t}', tag='gr', bufs=2)
                        act_raw(gr[:], gl[:], AF.Exp, scale=-1.0)
                        # one K=33 PE matmul broadcasts both hi rows of 1/d
                        # to [128, 512] (block-ones stationary)
                        rp2 = (mmp.tile([P, 512], f32, name=f'rp2{c}_{t}',
                                        tag='mm') if last else
                               fpsp.tile([P, 512], f32, name=f'rp2{c}_{t}',
                                         tag='fill'))
                        nc.tensor.matmul(rp2[:], r(ones2r[0:33, :]),
                                         r(gr[:]), start=True, stop=True)
                        if not last:
                            for hi in range(2):
                                nc.vector.tensor_mul(
                                    aoT_sb[t][64 * hi:64 * hi + 64,
                                              ts(c, 512)],
                                    pa[hi][:],
                                    rp2[64 * hi:64 * hi + 64, :])
                        else:
                            # multiply per 128-col m-block so each tail
                            # out_proj matmul unblocks as soon as its slice
                            # of aoT is normalized
                            for j in range(4):
                                for hi in range(2):
                                    nc.vector.tensor_mul(
                                        aoT_sb[t][64 * hi:64 * hi + 64,
                                                  512 * c + 128 * j:
                                                  512 * c + 128 * j + 128],
                                        pa[hi][:, 128 * j:128 * j + 128],
                                        rp2[64 * hi:64 * hi + 64,
                                            128 * j:128 * j + 128])
                                m = 4 * c + j
                                emit_oproj(m, 0,
                                           'act' if j % 2 == 0 else 'dve')
                                emit_oproj(m, 1,
                                           'dve' if j % 2 == 0 else 'act')
                if not final:
                    # chunk-level norm: one Ln/Exp for all 4 heads (cheapest
                    # on ACT), then one K=33 PE broadcast matmul per pair
                    # (block-ones stationary; ~330ns each, vs ~1.2us per
                    # serial gpsimd partition-broadcast), DVE muls
                    gl = rpool.tile([97, 512], f32, name=f'gl{c}', tag='gl',
                                    bufs=2)
                    act_raw(gl[:], g[:], AF.Ln)
                    gr = rpool.tile([97, 512], f32r, name=f'gr{c}', tag='gr',
                                    bufs=2)
                    act_raw(gr[:], gl[:], AF.Exp, scale=-1.0)
                    for t in range(2):
                        rp2 = fpsp.tile([P, 512], f32, name=f'rp2{c}_{t}',
                                        tag='fill')
                        nc.tensor.matmul(rp2[:],
                                         r(ones2r[64 * t:64 * t + 33, :]),
                                         r(gr[64 * t:64 * t + 33, :]),
                                         start=True, stop=True)
                        for hi in range(2):
                            nc.vector.tensor_mul(
                                aoT_sb[t][64 * hi:64 * hi + 64, ts(c, 512)],
                                aou[2 * t + hi][:],
                                rp2[64 * hi:64 * hi + 64, :])

            # ---------------- emission schedule ----------------
            # prefix: everything attention chunk 0 needs
            emit_qk('k', 0, 0)
            emit_qk('k', 1, 0)
            emit_qk('k', 0, 512)
            emit_qk('k', 1, 512)
            emit_v(0)
            emit_v(1)
            emit_qk('q', 0, 0)
            emit_qk('q', 1, 0)
            emit_qk('q', 0, 512)
            emit_qk('q', 1, 512)
            for sbb in (2, 3, 4, 5):
                emit_v(sbb)
            # per-head ones columns of v_sb (strided DVE copies; memset
            # cannot write bf16 and elementwise DMAs are pathological).
            # Emitted after the prefix so they don't head-block the DVE
            # queue while the ones DMA is in flight.
            for h in range(HPC):
                c0 = h * (HD + 1) + HD
                nc.vector.tensor_copy(
                    v_sb[:, :, c0:c0 + 1],
                    ones16_sb.rearrange('p (n o) -> p n o', o=1))

            # deferred projection work, drained inside attention loops.
            # att(0): fill with V 6..9 and kT blocks 8..11 (for att(1)).
            fillers += [lambda: emit_v(6), lambda: emit_v(7),
                        lambda: emit_qk('k', 0, 1024),
                        lambda: emit_qk('k', 1, 1024),
                        lambda: emit_v(8), lambda: emit_v(9)]
            emit_att(0, fill_at=[{0, 2, 4}, {0, 2, 4}])

            # att(1): fill with qT chunk-1, V 10..13 (for att(2))
            fillers += [lambda: emit_qk('q', 0, 1024),
                        lambda: emit_qk('q', 0, 1536),
                        lambda: emit_qk('q', 1, 1024),
                        lambda: emit_qk('q', 1, 1536),
                        lambda: emit_v(10), lambda: emit_v(11),
                        lambda: emit_v(12), lambda: emit_v(13)]
            emit_att(1, fill_at=[{0, 2, 4, 6}, {0, 2, 4, 6}])

            # kT tail blocks 12..15 + out_proj(0) + V 14,15 during att(2)
            fillers += [lambda: emit_qk('k', 0, 1536),
                        lambda: emit_qk('k', 1, 1536)]
            for m in range(4):
                for n in range(2):
                    fillers.append(
                        lambda m=m, n=n: emit_oproj(m, n, 'dve'))
            fillers += [lambda: emit_v(14), lambda: emit_v(15)]
            emit_att(2, fill_at=[{0, 2, 4, 6, 8, 10},
                                 {0, 2, 4, 6, 8, 10}])

            # out_proj(1) and out_proj(2) during att(3); out_proj(3) is
            # emitted inside emit_att's final per-m norm pipeline. All
            # stagings on DVE: an ACT staging would sit inside the exp
            # stream and delay the attention pacer.
            for m in range(4, 12):
                for n in range(2):
                    fillers.append(
                        lambda m=m, n=n: emit_oproj(m, n, 'dve'))
            emit_att(3, fill_at=[{0, 2, 4, 6, 8, 10, 12, 14},
                                 {0, 2, 4, 6, 8, 10, 12, 14}], final=True)
            drain(len(fillers))

    nc.compile()
    return nc


def _get_module():
    if 'nc' not in _MODULE_CACHE:
        _MODULE_CACHE['nc'] = _build_module()
    return _MODULE_CACHE['nc']


def _make_in_maps(x, in_proj_w, in_proj_b, out_proj_w):
    import ml_dtypes
    bf = ml_dtypes.bfloat16
    x = np.asarray(x, np.float32)
    in_proj_w = np.asarray(in_proj_w, np.float32)
    in_proj_b = np.asarray(in_proj_b, np.float32)
    out_proj_w = np.asarray(out_proj_w, np.float32)

    ND = D // P
    mask01b = (np.arange(P)[:, None] <= np.arange(P)[None, :])

    def swz_w(w):   # [DL, D] -> [P, ND, DL] (c-chunk-contiguous per partition)
        return np.ascontiguousarray(
            w.T.reshape(ND, P, DL).transpose(1, 0, 2)).astype(bf)

    xsw = []
    for b in range(B):
        xT = np.ascontiguousarray(x[b].T)          # [D, S]
        xsw.append(np.ascontiguousarray(
            xT.reshape(ND, P, 4, 512).transpose(1, 2, 0, 3)).astype(bf))

    in_maps = []
    for core in range(NCORES):
        b, hg = core // 4, core % 4
        sl = slice(DL * hg, DL * hg + DL)
        wq = in_proj_w[0 * D:1 * D][sl]
        wk = in_proj_w[1 * D:2 * D][sl]
        wv = in_proj_w[2 * D:3 * D][sl]
        bq = in_proj_b[0 * D:1 * D][sl]
        bk = in_proj_b[1 * D:2 * D][sl]
        bv = in_proj_b[2 * D:3 * D][sl]
        wo = out_proj_w[:, sl]                     # [D, DL]
        in_maps.append({
            'xs': xsw[b],
            'wqs': swz_w(wq),
            'wks': swz_w(wk),
            'wvs': swz_w(wv),
            'wos': np.ascontiguousarray(
                wo.T.reshape(2, P, D).transpose(1, 0, 2)).astype(bf),
            'bq2': np.ascontiguousarray((bq * SCALE).reshape(2, P).T),
            'bk2': np.ascontiguousarray(bk.reshape(2, P).T),
            'bvrow': bv.reshape(1, DL).copy(),
            'mask01': mask01b.astype(ml_dtypes.bfloat16),
            'onescol': np.ones((P, 16, 1), ml_dtypes.bfloat16),
        })
    return in_maps


def kernel(x, in_proj_w, in_proj_b, out_proj_w, out_proj_b):
    global LAST_RESULTS
    _install_ntff_shim()
    from concourse import bass_utils

    nc = _get_module()
    in_maps = _make_in_maps(x, in_proj_w, in_proj_b, out_proj_w)
    res = bass_utils.run_bass_kernel_spmd(
        nc, in_maps, core_ids=list(range(NCORES)),
        trace=TRACE,
        **({'trace_cores': TRACE_CORES} if TRACE_CORES else {}))
    LAST_RESULTS = res

    out = np.zeros((B, S, D), np.float32)
    for core in range(NCORES):
        out[core // 4] += np.asarray(res.results[core]['out'], np.float32)
    out += np.asarray(out_proj_b, np.float32)
    return out
